# revision 67
# baseline (speedup 1.0000x reference)
"""GravityField Trainium2 kernel.

out = U * sqrt(1 + clip(0.1 * grav, -0.9, 5) + 1e-6)
where grav[t] = phi[t] . sum_t'(phi[t'] * mass[t']), phi = sqrt(2/R)*cos(coords@W+b),
mass = softplus(relu(coords@w1+b1)@w2+b2).

Sharding: pure data-parallel over B (8 batches -> 8 cores, no communication).

v6 design (f16 I/O, bf16 feature plane, pair-packed partitions, host-folded
operands):
- Host feeds U f16 [T, 512] ("(p j)" interleave), coordsT f16 [65, T] with a
  ones row, the PRE-FOLDED stationary w_comb f16 [65,128] ([w1 | rffW/2pi]
  with bias row [b1 | (b+pi/2)/2pi]), w2 f16, identity f16, [I64;I64] bf16,
  and a [128, 5] consts plane (0, 1, 1+1e-6 sqrt bias, b2, MAGIC). No device
  memsets/transforms: first matmul fires as soon as w_comb + coords land.
- PE warm-up: ~30 dummy matmuls on w_comb release the HAM clock-gate
  throttle before the real pipeline starts.
- Phase M per 512-t chunk: one [65,128] f16 matmul -> h rows 0:64 + angle z
  rows 64:128 (PSUM); MAGIC range reduction (rru + fused stt -> fm bf16);
  rru and relu alternate DVE/scalar by chunk parity to balance both engines;
  mass pre-act via 4 [64,128]x[64,1] column matmuls into one PSUM bank.
- fm/phi live PAIR-PACKED [128, T/2] (chunk parity = partition half), so
  Sin runs [128,1024] tiles at full lane use and the phi_sum accumulate
  runs [128,·] bf16 stt at the fast DVE rate.
- mass tail: exp->ln softplus, PE transpose to parity-major rows, DRAM
  bounce, 8 half-broadcasts on 3 queues, 4 bf16 stt accumulates, then an
  [I64;I64] matmul folds the parity halves into phi_sum.
- Phase 2 streams per chunk: 4 strided-stationary grav matmuls, clip (DVE),
  sqrt (scalar), 3 DVE + 1 scalar in-place f16 multiplies, 0.5 MB
  out-writes on the sync queue.
"""

import sys

sys.path.insert(0, "/opt/trn_rl_repo")

import ml_dtypes
import numpy as np
from contextlib import ExitStack

import concourse.bass as bass
import concourse.bacc as bacc
import concourse.mybir as mybir
from concourse import tile
from concourse.bass_utils import run_bass_kernel_spmd

F32 = mybir.dt.float32
F16 = mybir.dt.float16
BF16 = mybir.dt.bfloat16
AF = mybir.ActivationFunctionType
ALU = mybir.AluOpType

B, T, D, R_LR, N_RFF = 8, 8192, 64, 8, 64
F = D * R_LR  # 512 f16 values of U per t
STRENGTH = 0.1
HALF_PI = 1.5707963267948966
INV_2PI = 0.15915494309189535
SIN_SCALE = 6.28318  # slightly under 2*pi: |SIN_SCALE * fm| <= 3.14159
MAGIC = 12582912.0  # 1.5 * 2**23: fp32 add rounds to nearest integer
# softplus(s) = relu(s) + P(u), u = min(|s|, 8): minimax degree-8 polynomial
# for ln(1+exp(-u)), max abs err 5.1e-5 (clamp tail adds <=3.4e-4) — replaces
# exp+ln and their activation-table loads with DVE ALU ops (no divide, which
# fails the cayman ISA check)
SP_C = [0.6931981133129991, -0.5004055182773283, 0.1247623530926259,
        0.0027674002837810015, -0.00911584654114335, 0.002377941393930663,
        -0.0003018420042302944, 1.971331200721323e-05, -5.298248843335719e-07]
PHI_SUM_SCALE = STRENGTH * 2.0 / N_RFF
BIGC = 512
N_BIG = T // BIGC  # 16
HT = T // 2  # packed feature width


def build_program():
    nc = bacc.Bacc("TRN2", target_bir_lowering=False, debug=False, num_devices=8)

    u_d = nc.dram_tensor("U", [T, F], F16, kind="ExternalInput")
    ct_d = nc.dram_tensor("coordsT", [D + 1, T], F16, kind="ExternalInput")
    wcomb_d = nc.dram_tensor("wcomb", [D + 1, 128], F16, kind="ExternalInput")
    w2_d = nc.dram_tensor("w2f16", [D, 1], F16, kind="ExternalInput")
    ident_d = nc.dram_tensor("ident", [128, 128], F16, kind="ExternalInput")
    i2_d = nc.dram_tensor("i2stack", [128, N_RFF], BF16, kind="ExternalInput")
    sel2_d = nc.dram_tensor("sel2", [2, 128], BF16, kind="ExternalInput")
    consts_d = nc.dram_tensor("consts", [128, 5], F32, kind="ExternalInput")
    out_d = nc.dram_tensor("out", [T, F], F16, kind="ExternalOutput")
    mscr_d = nc.dram_tensor("mscr", [2, HT], BF16)  # mass bounce, row = parity

    with tile.TileContext(nc) as tc, ExitStack() as ctx:
        const = ctx.enter_context(tc.tile_pool(name="const", bufs=1))

        # critical-path operands first on their queues
        w_comb = const.tile([D + 1, 128], F16)
        nc.scalar.dma_start(w_comb[:], wcomb_d[:, :])
        w2_sb = const.tile([D, 1], F16)
        nc.scalar.dma_start(w2_sb[:], w2_d[:, :])
        csb = const.tile([128, 5], F32)
        nc.scalar.dma_start(csb[:], consts_d[:, :])
        identity = const.tile([128, 128], F16)
        nc.scalar.dma_start(identity[:], ident_d[:, :])
        i2 = const.tile([128, N_RFF], BF16)
        nc.scalar.dma_start(i2[:], i2_d[:, :])
        sel2 = const.tile([2, 128], BF16)
        nc.scalar.dma_start(sel2[:], sel2_d[:, :])
        zero_b = csb[:, 0:1]
        one_b = csb[:, 1:2]
        sqrt_b = csb[:, 2:3]
        b2_b = csb[:, 3:4]
        magic_b = csb[:, 4:5]

        # first coords chunks ahead of U on BOTH queues so phase M starts ASAP
        ct_all = const.tile([D + 1, T], F16)
        ct_eng = [nc.sync, nc.gpsimd, nc.sync, nc.gpsimd]
        for q in range(4):
            qsl = slice(q * 2048, (q + 1) * 2048)
            ct_eng[q].dma_start(ct_all[:, qsl], ct_d[:, qsl])

        u_tiles = [
            const.tile([128, 4 * F], F16, name=f"u{i}") for i in range(N_BIG)
        ]
        # U preload: 16x 0.5 MB flat descriptors split over sync/gpsimd.
        # t-rows interleave "(p j)": partition p, col-block j <-> t = 512c+4p+j
        for c in range(N_BIG):
            tsl = slice(c * BIGC, (c + 1) * BIGC)
            eng = nc.sync if c % 2 == 0 else nc.gpsimd
            eng.dma_start(
                u_tiles[c][:],
                u_d[tsl, :].rearrange("(p j) f -> p (j f)", p=128),
            )

        fm_all = const.tile([128, HT], BF16)   # -frac, pair-packed
        phiT_all = const.tile([128, HT], BF16)  # -cos, pair-packed
        msp_cols = const.tile([128, 4 * N_BIG], F16)
        msp_rows = const.tile([4 * N_BIG, 128], BF16)
        mass_sb = const.tile([2, HT], BF16)
        partials = const.tile([128, 8], F32)
        acc_raw = const.tile([128, 1], F32)
        acc16 = const.tile([128, 1], BF16)
        phi_sum2 = const.tile([128, 1], BF16)  # phi_sum duplicated per half

        h_pool = ctx.enter_context(tc.tile_pool(name="hT", bufs=4))
        rr_pool = ctx.enter_context(tc.tile_pool(name="rr", bufs=4))
        prod_pool = ctx.enter_context(tc.tile_pool(name="prod", bufs=2))
        sc_pool = ctx.enter_context(tc.tile_pool(name="sc", bufs=3))

        with (
            tc.tile_pool(name="pbig", bufs=3, space=bass.MemorySpace.PSUM) as big_pool,
            tc.tile_pool(name="pma", bufs=1, space=bass.MemorySpace.PSUM) as ma_pool,
            tc.tile_pool(name="ptail", bufs=1, space=bass.MemorySpace.PSUM) as mt_pool,
            tc.tile_pool(name="pmb", bufs=2, space=bass.MemorySpace.PSUM) as mb_pool,
        ):
            # PE warm-up: dummy matmuls on w_comb release the HAM clock gate
            # (borrows an mb_pool rotation slot; result is never read)
            warm = mb_pool.tile([128, BIGC], F32, tag="mbp")
            for _ in range(30):
                nc.tensor.matmul(
                    warm[:, 0:128], w_comb[:], w_comb[:], start=True, stop=True
                )

            # mass pre-acts land as columns of ONE PSUM bank, parity-major:
            # col 32*(c%2) + 4*(c//2) + j holds pre(t = 512c + 128j + p)
            mTall = ma_pool.tile([128, 4 * N_BIG], F32, tag="mTall")
            hTs = {}
            for i in range(N_BIG + 1):
                # big(c) first on PE so its consumers unblock ASAP; the mass
                # matmuls of chunk c-1 fill the PE gap behind it
                if i < N_BIG:
                    c = i
                    tsl = slice(c * BIGC, (c + 1) * BIGC)
                    big = big_pool.tile([128, BIGC], F32, tag="big")
                    nc.tensor.matmul(
                        big[:], w_comb[:], ct_all[:, tsl], start=True, stop=True
                    )
                if 1 <= i:
                    cp = i - 1
                    hTp = hTs.pop(cp)
                    cbase = 32 * (cp % 2) + 4 * (cp // 2)
                    for j in range(4):
                        nc.tensor.matmul(
                            mTall[:, cbase + j : cbase + j + 1],
                            hTp[:, j * 128 : (j + 1) * 128],
                            w2_sb[:],
                            start=True, stop=True,
                        )
                if i < N_BIG:
                    half = slice(64 * (c % 2), 64 * (c % 2) + 64)
                    psl = slice((c // 2) * BIGC, (c // 2 + 1) * BIGC)
                    # range reduction: fm = round(z) - z, exact, |fm| <= 0.5.
                    # DVE is the phase-M bottleneck (fm-stt is DVE-only), so
                    # relu lives on scalar and rru goes to scalar 1-in-3
                    rru = rr_pool.tile([D, BIGC], F32, tag="rru")
                    hT = h_pool.tile([D, BIGC], F16, tag="hT")
                    if c % 8 == 7:
                        nc.vector.tensor_scalar_max(hT[:], big[0:D, :], 0.0)
                    else:
                        nc.scalar.activation(
                            hT[:], big[0:D, :], AF.Relu, bias=zero_b[0:D, :]
                        )
                    if c % 3 == 0:
                        nc.scalar.activation(
                            rru[:], big[D : 2 * D, :], AF.Copy, bias=MAGIC
                        )
                    else:
                        nc.vector.tensor_scalar_add(rru[:], big[D : 2 * D, :], MAGIC)
                    nc.vector.scalar_tensor_tensor(
                        fm_all[half, psl], rru[:], MAGIC, big[D : 2 * D, :],
                        op0=ALU.subtract, op1=ALU.subtract,
                    )
                    hTs[c] = hT
                    if c % 4 == 3:
                        ssl = slice((c // 4) * 1024, (c // 4 + 1) * 1024)
                        # phiT = sin(2pi*fm) = -cos(angle); sign cancels in grav
                        nc.scalar.activation(
                            phiT_all[:, ssl], fm_all[:, ssl], AF.Sin,
                            bias=zero_b[:], scale=SIN_SCALE,
                        )

            # mass = softplus(pre + b2) = relu(s) + P(min(|s|, 8)) on DVE and
            # scalar ALUs — no exp/ln table loads in the tail
            sps = const.tile([128, 4 * N_BIG], F32)
            nc.scalar.activation(sps[:], mTall[:], AF.Identity, bias=b2_b[:], scale=1.0)
            spr = const.tile([128, 4 * N_BIG], F16)
            nc.scalar.activation(spr[:], sps[:], AF.Relu, bias=zero_b[:])
            spa = const.tile([128, 4 * N_BIG], F32)
            nc.scalar.activation(spa[:], sps[:], AF.Abs)
            spu = const.tile([128, 4 * N_BIG], F32)
            nc.vector.tensor_scalar_min(spu[:], spa[:], 8.0)
            spp = const.tile([128, 4 * N_BIG], F32)
            nc.vector.tensor_scalar_mul(spp[:], spu[:], SP_C[8])
            for k in range(7, 0, -1):
                nc.vector.scalar_tensor_tensor(
                    spp[:], spp[:], SP_C[k], spu[:], op0=ALU.add, op1=ALU.mult
                )
            nc.vector.scalar_tensor_tensor(
                msp_cols[:], spp[:], SP_C[0], spr[:], op0=ALU.add, op1=ALU.add
            )
            mspT = mt_pool.tile([4 * N_BIG, 128], F16, tag="mspT")
            nc.tensor.transpose(mspT[:], msp_cols[:], identity[:])
            nc.scalar.copy(msp_rows[:], mspT[:])
            # bounce: parity-major rows 0:32 (even chunks) / 32:64 (odd)
            # through DRAM, then ONE small read back as [2, HT] — no slow
            # 64-partition broadcast DMAs
            nc.gpsimd.dma_start(
                mscr_d[0:1, :].rearrange("a (c p) -> (a c) p", c=32), msp_rows[0:32, :]
            )
            nc.sync.dma_start(
                mscr_d[1:2, :].rearrange("a (c p) -> (a c) p", c=32), msp_rows[32:64, :]
            )
            nc.gpsimd.dma_start(mass_sb[:], mscr_d[:, :])
            # phi_sum accumulate: PE broadcasts each parity row into the
            # matching partition half via the [2,128] selector, DVE does the
            # fused multiply+accumulate straight from PSUM
            for g in range(8):
                gsl = slice(g * 512, (g + 1) * 512)
                mbp = mb_pool.tile([128, 512], F32, tag="mbp")
                nc.tensor.matmul(
                    mbp[:], sel2[:], mass_sb[:, gsl], start=True, stop=True
                )
                prod = prod_pool.tile([128, 512], BF16, tag="prod")
                nc.vector.scalar_tensor_tensor(
                    prod[:], phiT_all[:, gsl], 1.0, mbp[:],
                    op0=ALU.mult, op1=ALU.mult,
                    accum_out=partials[:, g : g + 1],
                )
            nc.vector.reduce_sum(acc_raw[:], partials[:], axis=mybir.AxisListType.X)
            nc.vector.tensor_scalar_mul(acc16[:], acc_raw[:], PHI_SUM_SCALE)
            # fold parity halves: phi_sum = acc16[0:64] + acc16[64:128]
            pss = mt_pool.tile([N_RFF, 1], F32, tag="pss")
            nc.tensor.matmul(pss[:], i2[:], acc16[:], start=True, stop=True)
            nc.vector.tensor_copy(phi_sum2[0:64, :], pss[:])
            nc.vector.tensor_copy(phi_sum2[64:128, :], pss[:])

        with tc.tile_pool(name="pg", bufs=3, space=bass.MemorySpace.PSUM) as pg_pool:
            for g in range(N_BIG):
                tsl = slice(g * BIGC, (g + 1) * BIGC)
                half = slice(64 * (g % 2), 64 * (g % 2) + 64)
                psl = slice((g // 2) * BIGC, (g // 2 + 1) * BIGC)
                # influence columns: pg[p, j] = 0.1*grav(t = 512g + 4p + j)
                # via strided stationary view of the packed phi plane
                phiT_perm = phiT_all[half, psl].rearrange("r (p j) -> r j p", p=128)
                pg = pg_pool.tile([128, 4], F32, tag="pg")
                for j in range(4):
                    nc.tensor.matmul(
                        pg[:, j : j + 1],
                        phiT_perm[:, j],
                        phi_sum2[half, :],
                        start=True, stop=True,
                    )
                infl = sc_pool.tile([128, 4], F32, tag="infl")
                nc.vector.tensor_scalar(
                    infl[:], pg[:], -0.9, 5.0, op0=ALU.max, op1=ALU.min
                )
                sc4 = sc_pool.tile([128, 4], F32, tag="sc4")
                nc.scalar.activation(sc4[:], infl[:], AF.Sqrt, bias=sqrt_b[:])

                ut = u_tiles[g]
                for j in range(4):
                    usl = slice(j * F, (j + 1) * F)
                    if j == 3:
                        nc.scalar.mul(ut[:, usl], ut[:, usl], sc4[:, j : j + 1])
                    else:
                        nc.vector.tensor_scalar_mul(
                            ut[:, usl], ut[:, usl], sc4[:, j : j + 1]
                        )
                nc.sync.dma_start(
                    out_d[tsl, :].rearrange("(p j) f -> p (j f)", p=128), ut[:]
                )

    nc.compile()
    return nc


_NC_CACHE = None


def _get_program():
    global _NC_CACHE
    if _NC_CACHE is None:
        _NC_CACHE = build_program()
    return _NC_CACHE


def run(inputs: dict, trace: bool = False, tmpdir=None):
    nc = _get_program()
    BF = ml_dtypes.bfloat16
    U = np.asarray(inputs["U"], dtype=np.float32).reshape(B, T, F).astype(np.float16)
    coords = np.asarray(inputs["coords"], dtype=np.float32)
    coordsT = np.concatenate(
        [coords.transpose(0, 2, 1), np.ones((B, 1, T), np.float32)], axis=1
    ).astype(np.float16)

    w1 = np.asarray(inputs["mass_w1"], np.float32)
    b1 = np.asarray(inputs["mass_b1"], np.float32)
    w2 = np.asarray(inputs["mass_w2"], np.float32)
    b2 = np.asarray(inputs["mass_b2"], np.float32)
    rffW = np.asarray(inputs["rff_W"], np.float32)
    rffb = np.asarray(inputs["rff_b"], np.float32)
    wl = np.concatenate([w1, rffW * INV_2PI], axis=1)
    bl = np.concatenate([b1, (rffb + HALF_PI) * INV_2PI])
    wcomb = np.concatenate([wl, bl[None, :]], axis=0).astype(np.float16)
    consts = np.zeros((128, 5), np.float32)
    consts[:, 1] = 1.0
    consts[:, 2] = 1.000001
    consts[:, 3] = b2[0]
    consts[:, 4] = MAGIC
    ident = np.eye(128, dtype=np.float16)
    i2 = np.concatenate([np.eye(64), np.eye(64)], axis=0).astype(BF)
    sel2 = np.zeros((2, 128), np.float32)
    sel2[0, 0:64] = 1.0
    sel2[1, 64:128] = 1.0

    shared = {
        "wcomb": wcomb,
        "w2f16": w2.astype(np.float16),
        "ident": ident,
        "i2stack": i2,
        "sel2": sel2.astype(BF),
        "consts": consts,
    }
    in_maps = [
        {"U": np.ascontiguousarray(U[i]), "coordsT": np.ascontiguousarray(coordsT[i]),
         **shared}
        for i in range(B)
    ]
    res = run_bass_kernel_spmd(nc, in_maps, list(range(B)), trace=trace, tmpdir=tmpdir)
    out = np.stack([res.results[i]["out"].reshape(T, D, R_LR) for i in range(B)])
    return out.astype(np.float32), res


def kernel(**inputs) -> np.ndarray:
    out, _ = run(inputs, trace=False)
    return out
